# revision 1
# baseline (speedup 1.0000x reference)
"""Trainium2 Bass kernel for a DETR-style Hungarian box matcher.

kernel(out_boxes, tgt_boxes) -> [32, 2, 128] int32.
Batch-parallel over 8 NeuronCores (4 images per core).
"""
import numpy as np
import concourse.bass as bass
import concourse.mybir as mybir
import concourse.tile as tile
from concourse.alu_op_type import AluOpType
from concourse._compat import with_exitstack

dt = mybir.dt
AX = mybir.AxisListType

NIMG = 4
M = 128          # targets per image
N = 2048         # preds per image
BIG = 1e30
NBIG = -1e30
CP = 32          # solver partition count (cols as [32, 64])
CF = N // CP     # 64
MAXIT = 16       # Dijkstra inner-iteration cap (measured max 5)
SHIFT = 2.0      # cost shift to keep everything positive
REVC = 4096.0    # j encoded as REVC - j for first-match argmin


def build(tc, ctx, outs, ins, dbg=None):
    nc = tc.nc
    ob_d = ins["ob"]   # [NIMG, 2048, 4]
    tb_d = ins["tb"]   # [NIMG, 128, 4]
    out_d = outs["out"]  # [NIMG, 2, 128] int32

    pool = ctx.enter_context(tc.tile_pool(name="main", bufs=1))
    cpool = ctx.enter_context(tc.tile_pool(name="cost", bufs=1))

    # ---------------- constants ----------------
    iotaJ = pool.tile([CP, CF], dt.int32)      # j = p*64 + f
    iotaJf = pool.tile([CP, CF], dt.float32)
    niotaJf = pool.tile([CP, CF], dt.float32)
    revJf = pool.tile([CP, CF], dt.float32)    # REVC - j
    rowiota = pool.tile([M, 1], dt.int32)
    rowiotaF = pool.tile([M, 1], dt.float32)   # i
    rowiotaP1 = pool.tile([M, 1], dt.float32)  # i+1
    iota128f = pool.tile([M, M], dt.float32)   # k along free
    f32tab = pool.tile([1, M], dt.float32)     # float(k)
    ones128 = pool.tile([1, M], dt.float32)
    ones_f = pool.tile([CP, 1], dt.float32)
    bigt = pool.tile([CP, CF], dt.float32)
    ones01 = pool.tile([CP, CF], dt.float32)

    nc.gpsimd.iota(iotaJ[:], pattern=[[1, CF]], base=0, channel_multiplier=CF)
    nc.vector.tensor_copy(iotaJf[:], iotaJ[:])
    nc.vector.tensor_scalar_mul(niotaJf[:], iotaJf[:], -1.0)
    nc.vector.tensor_scalar(revJf[:], iotaJf[:], -1.0, REVC,
                            AluOpType.mult, AluOpType.add)
    nc.gpsimd.iota(rowiota[:], pattern=[[1, 1]], base=0, channel_multiplier=1)
    nc.vector.tensor_copy(rowiotaF[:], rowiota[:])
    nc.vector.tensor_scalar_add(rowiotaP1[:], rowiotaF[:], 1.0)
    nc.gpsimd.iota(iota128f[:], pattern=[[1, M]], base=0, channel_multiplier=0,
                   allow_small_or_imprecise_dtypes=True)
    nc.gpsimd.iota(f32tab[:].bitcast(dt.int32), pattern=[[1, M]], base=0,
                   channel_multiplier=0)
    nc.vector.tensor_copy(f32tab[:], f32tab[:].bitcast(dt.int32))
    eye128 = pool.tile([M, M], dt.float32)
    nc.vector.memset(ones128[:], 1.0)
    nc.vector.memset(ones_f[:], 1.0)
    nc.vector.memset(bigt[:], BIG)
    nc.vector.memset(ones01[:], 1.0)
    nc.vector.tensor_scalar(eye128[:], iota128f[:], rowiotaF[:], None,
                            AluOpType.is_equal)

    # ---------------- phase 1: cost build ----------------
    from contextlib import ExitStack as _ES
    p1ctx = _ES()
    p1pool = p1ctx.enter_context(tc.tile_pool(name="p1", bufs=1))
    # per-target scalars (areaT persists: used in chunk ops only -> p1)
    tgt = [p1pool.tile([M, 4], dt.float32, tag=f"tgt{g}", name=f"tgt{g}") for g in range(NIMG)]
    areaT = [p1pool.tile([M, 1], dt.float32, tag=f"areaT{g}", name=f"areaT{g}") for g in range(NIMG)]
    # pred-partition layout for area compute: [128, 16, 4]
    obp = p1pool.tile([M, N // M, 4], dt.float32)
    areaP = p1pool.tile([M, N // M], dt.float32)
    wP = p1pool.tile([M, N // M], dt.float32)
    hP = p1pool.tile([M, N // M], dt.float32)

    cost_A = [cpool.tile([M, N], dt.float32, tag=f"cost_A{g}", name=f"cost_A{g}") for g in range(NIMG)]
    obrow = [p1pool.tile([1, 4 * N], dt.float32, tag="obrow",
                          name=f"obrow{g}") for g in range(NIMG)]
    arearow = [p1pool.tile([1, N], dt.float32, tag="arearow",
                           name=f"arearow{g}") for g in range(NIMG)]

    for g in range(NIMG):
        nc.sync.dma_start(tgt[g][:], tb_d[g, :, :])
        # tgt area (+eps)
        tw = pool.tile([M, 1], dt.float32, tag="tw")
        th = pool.tile([M, 1], dt.float32, tag="th")
        nc.vector.tensor_sub(tw[:], tgt[g][:, 2:3], tgt[g][:, 0:1])
        nc.vector.tensor_sub(th[:], tgt[g][:, 3:4], tgt[g][:, 1:2])
        ta = pool.tile([M, 1], dt.float32, tag="ta")
        nc.vector.tensor_tensor(ta[:], tw[:], th[:], AluOpType.mult)
        nc.vector.tensor_scalar_add(areaT[g][:], ta[:], 1e-8)

        # pred coords flat on one partition + areas; broadcast later via PE
        nc.sync.dma_start(obrow[g][:], ob_d[g].rearrange("n c -> (n c)")
                          .rearrange("(a n) -> a n", a=1))
        nc.sync.dma_start(obp[:], ob_d[g].rearrange("(p i) c -> p i c", p=M))
        nc.vector.tensor_sub(wP[:], obp[:, :, 2], obp[:, :, 0])
        nc.vector.tensor_sub(hP[:], obp[:, :, 3], obp[:, :, 1])
        nc.vector.tensor_tensor(areaP[:], wP[:], hP[:], AluOpType.mult)
        nc.sync.dma_start(arearow[g][:], areaP[:])

    # broadcast slabs + dense ops, chunked over preds
    CHUNK = 512
    NCH = N // CHUNK
    bpool = p1ctx.enter_context(tc.tile_pool(name="bcast", bufs=1,
                                              space="PSUM"))
    tpool = p1ctx.enter_context(tc.tile_pool(name="ctmp", bufs=2))
    for g in range(NIMG):
        x1s, y1s, x2s, y2s = (tgt[g][:, c:c + 1] for c in range(4))
        for ch in range(NCH):
            slab = bpool.tile([M, CHUNK, 4], dt.float32, tag="slab")
            slaba = bpool.tile([M, CHUNK], dt.float32, tag="slaba")
            # broadcast via PE: ones[128] (x) row -> PSUM (FD<=512 per matmul)
            Q = CHUNK // 4
            for q in range(4):
                nc.tensor.matmul(
                    slab[:, q * Q:(q + 1) * Q, :],
                    ones128[:],
                    obrow[g][0:1, ch * CHUNK * 4 + q * Q * 4:
                             ch * CHUNK * 4 + (q + 1) * Q * 4])
            nc.tensor.matmul(slaba[:], ones128[:],
                             arearow[g][0:1, ch * CHUNK:(ch + 1) * CHUNK])
            xb1 = slab[:, :, 0]
            yb1 = slab[:, :, 1]
            xb2 = slab[:, :, 2]
            yb2 = slab[:, :, 3]
            areab = slaba[:]

            t = lambda tag: tpool.tile([M, CHUNK], dt.float32, tag=tag, name=tag)
            ltx, lty, rbx, rby = t("ltx"), t("lty"), t("rbx"), t("rby")
            eltx, elty, erbx, erby = t("eltx"), t("elty"), t("erbx"), t("erby")
            nc.vector.tensor_scalar_max(ltx[:], xb1, x1s)
            nc.vector.tensor_scalar_max(lty[:], yb1, y1s)
            nc.vector.tensor_scalar_min(rbx[:], xb2, x2s)
            nc.vector.tensor_scalar_min(rby[:], yb2, y2s)
            nc.vector.tensor_scalar_min(eltx[:], xb1, x1s)
            nc.vector.tensor_scalar_min(elty[:], yb1, y1s)
            nc.vector.tensor_scalar_max(erbx[:], xb2, x2s)
            nc.vector.tensor_scalar_max(erby[:], yb2, y2s)
            wxp, wyp, ex, ey = t("wxp"), t("wyp"), t("ex"), t("ey")
            nc.vector.tensor_sub(wxp[:], rbx[:], ltx[:])
            nc.vector.tensor_sub(wyp[:], rby[:], lty[:])
            nc.vector.tensor_sub(ex[:], erbx[:], eltx[:])
            nc.vector.tensor_sub(ey[:], erby[:], elty[:])
            wx, wy = t("wx"), t("wy")
            nc.scalar.activation(wx[:], wxp[:], mybir.ActivationFunctionType.Relu)
            nc.scalar.activation(wy[:], wyp[:], mybir.ActivationFunctionType.Relu)
            inter, union_eps, runion, iou = t("inter"), t("union"), t("runion"), t("iou")
            nc.vector.tensor_tensor(inter[:], wx[:], wy[:], AluOpType.mult)
            # union+eps = (areab + areaT') - inter   (areaT' has +1e-8)
            nc.vector.scalar_tensor_tensor(union_eps[:], areab, areaT[g][:],
                                           inter[:], AluOpType.add,
                                           AluOpType.subtract)
            nc.vector.reciprocal_approx_fast(runion[:], union_eps[:])
            nc.vector.tensor_tensor(iou[:], inter[:], runion[:], AluOpType.mult)
            earea, rearea, tq, q = t("earea"), t("rearea"), t("tq"), t("q")
            eare0 = t("eare0")
            nc.vector.tensor_tensor(eare0[:], ex[:], ey[:], AluOpType.mult)
            nc.vector.tensor_scalar_add(earea[:], eare0[:], 1e-8)
            nc.vector.reciprocal_approx_fast(rearea[:], earea[:])
            nc.vector.tensor_sub(tq[:], earea[:], union_eps[:])
            nc.vector.tensor_tensor(q[:], tq[:], rearea[:], AluOpType.mult)
            l1a, l1b, c1, c2 = t("l1a"), t("l1b"), t("c1"), t("c2")
            nc.vector.tensor_add(l1a[:], ex[:], ey[:])
            nc.vector.tensor_add(l1b[:], wxp[:], wyp[:])
            nc.vector.tensor_sub(c1[:], l1a[:], l1b[:])
            nc.vector.tensor_sub(c2[:], c1[:], iou[:])
            # cost = c2 + q + SHIFT
            nc.vector.scalar_tensor_tensor(
                cost_A[g][:, ch * CHUNK:(ch + 1) * CHUNK],
                q[:], SHIFT, c2[:], AluOpType.add, AluOpType.add)

    p1ctx.close()
    if dbg is not None and "cost" in dbg:
        for g in range(NIMG):
            nc.sync.dma_start(dbg["cost"][g], cost_A[g][:])
    return pool, cpool, cost_A, dict(
        iotaJf=iotaJf, niotaJf=niotaJf, revJf=revJf,
        rowiotaF=rowiotaF, rowiotaP1=rowiotaP1, iota128f=iota128f,
        f32tab=f32tab, ones128=ones128, ones_f=ones_f, bigt=bigt,
        ones01=ones01, eye128=eye128)


def build_solve(tc, ctx, outs, ins, pool, cpool, cost_A, C, dbg=None):
    """Phases 2-4: prepass claim, residual JV solve, output sort."""
    nc = tc.nc
    out_d = outs["out"]
    v = nc.vector

    iotaJf, niotaJf, revJf = C["iotaJf"], C["niotaJf"], C["revJf"]
    rowiotaF, rowiotaP1 = C["rowiotaF"], C["rowiotaP1"]
    iota128f, f32tab, ones128 = C["iota128f"], C["f32tab"], C["ones128"]
    eye128 = C["eye128"]
    ones_f, bigt, ones01 = C["ones_f"], C["bigt"], C["ones01"]

    from contextlib import ExitStack as _ES
    spool = ctx.enter_context(tc.tile_pool(name="solve", bufs=1))
    p2ctx = _ES()
    ppool = p2ctx.enter_context(tc.tile_pool(name="prep", bufs=1))
    pspool = p2ctx.enter_context(tc.tile_pool(name="ps", bufs=1, space="PSUM"))
    iota2048f = ppool.tile([M, N], dt.float32)
    nc.gpsimd.iota(iota2048f[:], pattern=[[1, N]], base=0, channel_multiplier=0,
                   allow_small_or_imprecise_dtypes=True)

    # extra consts for prepass
    revRow = pool.tile([M, 1], dt.float32)     # 128 - i
    nc.vector.tensor_scalar(revRow[:], rowiotaF[:], -1.0, float(M),
                            AluOpType.mult, AluOpType.add)
    ones11 = pool.tile([1, 1], dt.float32)
    nc.vector.memset(ones11[:], 1.0)

    # ---------------- phase 2: prepass (per image) ----------------
    NRES = 16
    # per-image persistent tiles
    u_A = [spool.tile([1, M], dt.float32, tag=f"u_A{g}", name=f"u_A{g}")
           for g in range(NIMG)]
    row4col = [spool.tile([1, N], dt.int32, tag=f"row4col{g}", name=f"row4col{g}")
               for g in range(NIMG)]
    col4row = [spool.tile([1, M], dt.int32, tag=f"col4row{g}", name=f"col4row{g}")
               for g in range(NIMG)]
    reslist = [spool.tile([1, NRES], dt.uint32, tag=f"reslist{g}", name=f"reslist{g}")
               for g in range(NIMG)]
    ressc = [spool.tile([1, NRES], dt.int32, tag=f"ressc{g}", name=f"ressc{g}")
             for g in range(NIMG)]

    for g in range(NIMG):
        neg = ppool.tile([M, N], dt.float32, tag="scr2048", name="neg")
        nc.vector.tensor_scalar_mul(neg[:], cost_A[g][:], -1.0)
        nm8 = ppool.tile([M, 8], dt.float32, tag="nm8")
        i8 = ppool.tile([M, 8], dt.uint32, tag="i8")
        nc.vector.max(nm8[:], neg[:])
        nc.vector.max_index(i8[:], nm8[:], neg[:])
        j1f = ppool.tile([M, 1], dt.float32, tag="j1f")
        nc.vector.tensor_copy(j1f[:], i8[:, 0:1])
        uval = ppool.tile([M, 1], dt.float32, tag="uval")
        nc.vector.tensor_scalar_mul(uval[:], nm8[:, 0:1], -1.0)

        # pairwise conflict: j1 along free via PE
        j1rowp = pspool.tile([1, M], dt.float32, tag="j1rowp")
        nc.tensor.matmul(j1rowp[:], j1f[:], eye128[:])
        j1row = ppool.tile([1, M], dt.float32, tag="j1row")
        nc.vector.tensor_copy(j1row[:], j1rowp[:])
        j1b = pspool.tile([M, M], dt.float32, tag="j1b")
        nc.tensor.matmul(j1b[:], ones128[:], j1row[:])
        eqm = ppool.tile([M, M], dt.float32, tag="eqm")
        nc.vector.tensor_scalar(eqm[:], j1b[:], j1f[:], None, AluOpType.is_equal)
        csum = ppool.tile([M, 1], dt.float32, tag="csum")
        nc.vector.tensor_reduce(csum[:], eqm[:], AX.X, AluOpType.add)
        assigned = ppool.tile([M, 1], dt.float32, tag="assigned")
        nc.vector.tensor_scalar(assigned[:], csum[:], 1.0, None, AluOpType.is_equal)

        # col4row (j+1, 0 if unassigned) per-row; u; residual score
        c4r_f = ppool.tile([M, 1], dt.float32, tag="c4r_f")
        nc.vector.scalar_tensor_tensor(c4r_f[:], j1f[:], 1.0, assigned[:],
                                       AluOpType.add, AluOpType.mult)
        notass = ppool.tile([M, 1], dt.float32, tag="notass")
        nc.vector.tensor_scalar(notass[:], assigned[:], -1.0, 1.0,
                                AluOpType.mult, AluOpType.add)
        resscore = ppool.tile([M, 1], dt.float32, tag="resscore")
        nc.vector.tensor_tensor(resscore[:], notass[:], revRow[:], AluOpType.mult)

        # transpose [128,1] columns to [1,128] rows via PE (one matmul each;
        # M=1 keeps every result on partition 0)
        c4rT = pspool.tile([1, M], dt.float32, tag="c4rT")
        nc.tensor.matmul(c4rT[:], c4r_f[:], eye128[:])
        uT = pspool.tile([1, M], dt.float32, tag="uT")
        nc.tensor.matmul(uT[:], uval[:], eye128[:])
        resT = pspool.tile([1, M], dt.float32, tag="resT")
        nc.tensor.matmul(resT[:], resscore[:], eye128[:])
        nc.vector.tensor_copy(col4row[g][:], c4rT[:])
        nc.vector.tensor_copy(u_A[g][:], uT[:])
        resrow = ppool.tile([1, M], dt.float32, tag="resrow")
        nc.vector.tensor_copy(resrow[:], resT[:])

        # row4col via onehot matmul: lhsT = (i+1)*assigned
        rpa = ppool.tile([M, 1], dt.float32, tag="rpa")
        nc.vector.tensor_tensor(rpa[:], rowiotaP1[:], assigned[:], AluOpType.mult)
        onehot = ppool.tile([M, N], dt.float32, tag="scr2048", name="onehot")
        nc.vector.tensor_scalar(onehot[:], iota2048f[:], j1f[:], None,
                                AluOpType.is_equal)
        for chk in range(4):
            r4cp = pspool.tile([1, 512], dt.float32, tag="r4cp")
            nc.tensor.matmul(r4cp[:], rpa[:], onehot[:, chk * 512:(chk + 1) * 512])
            nc.vector.tensor_copy(row4col[g][0:1, chk * 512:(chk + 1) * 512],
                                  r4cp[:])

        # residual list: two rounds of max8 + match_replace
        rv1 = ppool.tile([1, 8], dt.float32, tag="rv1")
        ri1 = ppool.tile([1, 8], dt.uint32, tag="ri1")
        nc.vector.max(rv1[:], resrow[:])
        nc.vector.max_index(ri1[:], rv1[:], resrow[:])
        rz = ppool.tile([1, M], dt.float32, tag="rz")
        nc.vector.match_replace(rz[:], rv1[:], resrow[:], 0.0)
        rv2 = ppool.tile([1, 8], dt.float32, tag="rv2")
        ri2 = ppool.tile([1, 8], dt.uint32, tag="ri2")
        nc.vector.max(rv2[:], rz[:])
        nc.vector.max_index(ri2[:], rv2[:], rz[:])
        nc.vector.tensor_copy(reslist[g][0:1, 0:8], ri1[:])
        nc.vector.tensor_copy(reslist[g][0:1, 8:16], ri2[:])
        nc.vector.tensor_copy(ressc[g][0:1, 0:8], rv1[:])
        nc.vector.tensor_copy(ressc[g][0:1, 8:16], rv2[:])


    p2ctx.close()
    if dbg is not None:
        for g in range(NIMG):
            if "u0" in dbg and g == 0:
                nc.sync.dma_start(dbg["u0"], u_A[0][:])
                nc.sync.dma_start(dbg["row4col0"], row4col[0][:])
                nc.sync.dma_start(dbg["col4row0"], col4row[0][:])
                nc.sync.dma_start(dbg["reslist0"], reslist[0][:])
                nc.sync.dma_start(dbg["ressc0"], ressc[0][:])
    return dict(u_A=u_A, row4col=row4col, col4row=col4row, reslist=reslist,
                ressc=ressc, spool=spool, ones11=ones11)


def build_residual(tc, ctx, outs, ins, pool, cpool, cost_A, C, S, dbg=None):
    """Phase 3: residual JV Dijkstra per image, DVE + its sequencer only."""
    nc = tc.nc
    ii = nc.vector
    out_d = outs["out"]
    NRES = 16

    def VL(ap, lo=None, hi=None):
        # reg_load + snap: bounded load WITHOUT the SeqAssert opcode
        # (SeqAssert is emitted by value_load's s_assert_within and
        # crashes real hardware)
        rg = ii.alloc_register(f"vl_{nc.next_id()}")
        ii.reg_load(rg, ap)
        return ii.snap(rg, donate=True, min_val=lo, max_val=hi)

    iotaJf, niotaJf, revJf = C["iotaJf"], C["niotaJf"], C["revJf"]
    f32tab, ones_f, bigt, ones01 = C["f32tab"], C["ones_f"], C["bigt"], C["ones01"]
    u_A, row4col, col4row = S["u_A"], S["row4col"], S["col4row"]
    reslist, ressc, spool = S["reslist"], S["ressc"], S["spool"]

    cbpool = ctx.enter_context(tc.tile_pool(name="cbp", bufs=1))

    # solver scratch tiles (shared across images via tags)
    def st(name, shape, dtt=dt.float32):
        return spool.tile(shape, dtt, tag=name, name=name)

    spc = st("spc", [CP, CF])          # true shortest-path costs (frozen)
    spcm = st("spcm", [CP, CF])        # argmin array (scanned -> +BIG)
    stamp = st("stamp", [CP, CF])      # iteration idx that last improved j
    v_eff = st("v_eff", [CP, CF])      # v - BIG*scanned
    red = st("red", [CP, CF])
    better = st("better", [CP, CF], dt.int32)
    eqm2 = st("eqm2", [CP, CF])
    score = st("score", [CP, CF])
    ohtmp = st("ohtmp", [CP, CF])
    ohBIG = st("ohBIG", [CP, CF])
    o1 = st("o1", [CP, CF])
    o2 = st("o2", [CP, CF])
    se = st("se", [CP, CF])
    sc01 = st("sc01", [CP, CF])
    vsp = st("vsp", [CP, CF])
    mscr = st("mscr", [CP, CP])        # -BIG filled, row0 = bcast src
    mscr2 = st("mscr2", [CP, CP])
    pscr = st("pscr", [CP, CP])
    pmin = st("pmin", [CP, 1])
    gmin_rep = st("gmin_rep", [CP, 1])
    d_rep = st("d_rep", [CP, 1])
    k_rep = st("k_rep", [CP, 1])
    srow = st("srow", [CP, 1])
    smax_rep = st("smax_rep", [CP, 1])
    d_sb = st("d_sb", [CP, 1])
    kf_sb = st("kf_sb", [CP, 1])
    minv_sb = st("minv_sb", [CP, 1])
    bfly = st("bfly", [CP, 1])
    jrep = st("jrep", [CP, 1])
    jfc = st("jfc", [CP, 1])
    jfi = st("jfi", [CP, 1], dt.int32)
    jconv = st("jconv", [1, 1], dt.int32)
    kconv = st("kconv", [1, 1], dt.int32)
    tdif = st("tdif", [1, 1])
    mt_sb = st("mt_sb", [1, 1])
    k_sb = st("k_sb", [1, 1], dt.int32)
    cur_sb = st("cur_sb", [1, 1], dt.int32)
    s_sb = st("s_sb", [1, 1], dt.int32)
    t_sb = st("t_sb", [1, 1], dt.int32)
    j_sb = st("j_sb", [1, 1], dt.int32)
    curlist = st("curlist", [1, MAXIT + 1], dt.int32)
    minvlist = st("minvlist", [1, MAXIT + 1], dt.int32)
    ovf = st("ovf", [1, NIMG], dt.int32)

    nc.vector.memset(mscr[:], NBIG)
    nc.vector.memset(mscr2[:], NBIG)
    nc.vector.memset(ovf[:], 0)
    nc.vector.memset(d_sb[:], 0.0)
    nc.vector.memset(kf_sb[:], 0.0)
    nc.vector.memset(minv_sb[:], 0.0)
    nc.vector.memset(bfly[:], 0.0)
    nc.vector.memset(jrep[:], 0.0)
    nc.vector.memset(jfc[:], 0.0)
    nc.vector.memset(jfi[:], 0)
    BCAST0 = [0] * CP
    XMASKS = [[p ^ (1 << b) for p in range(CP)] for b in (4, 3, 2, 1, 0)]

    def butterfly(ii, tile_, op):
        """All-reduce over 32 partitions of [32,1] tile_, result replicated."""
        for mk in XMASKS:
            ii.stream_shuffle(bfly[:], tile_[:], mk)
            ii.drain()
            ii.tensor_tensor(tile_[:], tile_[:], bfly[:], op)
            ii.drain()



    c_Bs = []
    for g in range(NIMG):
        c_Bg = cbpool.tile([CP, M * CF], dt.float32, tag=f"cB{g}", name=f"c_B{g}")
        for p in range(CP):
            nc.scalar.dma_start(c_Bg[p:p + 1, :],
                                cost_A[g][:, p * CF:(p + 1) * CF])
        c_Bs.append(c_Bg)

    with tc.tile_critical():
      for g in range(NIMG):
        c_Bg = c_Bs[g]
        if True:
            # v starts at 0 for each image
            ii.memset(v_eff[:], 0.0)
            ii.drain()
            sleft = ii.alloc_register(f"sleft{g}")
            cont = ii.alloc_register(f"cont{g}")
            cont2 = ii.alloc_register(f"cont2{g}")
            sink_r = ii.alloc_register(f"sink{g}")
            i_r = ii.alloc_register(f"ir{g}")
            ii.reg_mov(sleft, NRES)
            ii.reg_save(s_sb[:], 0)
            with ii.While(lambda: sleft):
                sv = VL(s_sb[:], 0, NRES - 1)
                scv = VL(ressc[g][0:1, bass.ds(sv, 1)])
                scr_reg = ii.to_reg(scv)
                with ii.If_cmp(scr_reg, 0, "IS_NE"):
                    iv = VL(
                        reslist[g][0:1, bass.ds(sv, 1)].bitcast(dt.int32), 0, M - 1)
                    ii.reg_mov(i_r, iv)
                    # ---- Dijkstra from row iv ----
                    ii.memset(spc[:], BIG)
                    ii.memset(spcm[:], BIG)
                    ii.memset(stamp[:], 0.0)
                    ii.memset(minv_sb[:], 0.0)
                    ii.drain()
                    ii.reg_save(k_sb[:], 0)
                    ii.reg_save(cur_sb[:], iv)
                    ii.reg_save(curlist[0:1, 0:1], iv)
                    ii.reg_mov(cont, 1)
                    with ii.While(lambda: cont):
                        kv = VL(k_sb[:], 0, MAXIT - 1)
                        curv = VL(cur_sb[:], 0, M - 1)
                        kb = VL(f32tab[0:1, bass.ds(kv, 1)]
                                           .bitcast(dt.int32))
                        ii.reg_save(kf_sb[0:1, 0:1].bitcast(dt.int32), kb)
                        # d = minv - u[cur] at partition 0, then bcast
                        ii.tensor_tensor(d_sb[0:1, 0:1], minv_sb[0:1, 0:1],
                                         u_A[g][0:1, bass.ds(curv, 1)],
                                         AluOpType.subtract)
                        ii.drain()
                        ii.stream_shuffle(d_rep[:], d_sb[:], BCAST0)
                        ii.stream_shuffle(k_rep[:], kf_sb[:], BCAST0)
                        ii.drain()
                        # relax: red = (c_row + d_rep) - v_eff
                        ii.scalar_tensor_tensor(
                            red[:], c_Bg[:, bass.ds(curv * CF, CF)], d_rep[:],
                            v_eff[:], AluOpType.add, AluOpType.subtract)
                        ii.drain()
                        ii.tensor_tensor(better[:], red[:], spc[:],
                                         AluOpType.is_lt)
                        ii.drain()
                        ii.tensor_tensor(spc[:], spc[:], red[:], AluOpType.min)
                        ii.tensor_tensor(spcm[:], spcm[:], red[:], AluOpType.min)
                        ii.drain()
                        ii.copy_predicated(stamp[:], better[:],
                                           k_rep[:].to_broadcast((CP, CF)))
                        ii.tensor_reduce(pmin[:], spcm[:], AX.X, AluOpType.min)
                        ii.drain()
                        ii.tensor_copy(gmin_rep[:], pmin[:])
                        ii.drain()
                        butterfly(ii, gmin_rep, AluOpType.min)
                        ii.tensor_scalar(eqm2[:], spcm[:], gmin_rep[:], None,
                                         AluOpType.is_equal)
                        ii.tensor_copy(minv_sb[0:1, 0:1], gmin_rep[0:1, 0:1])
                        ii.drain()
                        ii.tensor_tensor(score[:], eqm2[:], revJf[:],
                                         AluOpType.mult)
                        ii.drain()
                        ii.tensor_reduce(srow[:], score[:], AX.X, AluOpType.max)
                        ii.drain()
                        ii.tensor_copy(smax_rep[:], srow[:])
                        ii.drain()
                        butterfly(ii, smax_rep, AluOpType.max)
                        ii.tensor_copy(jconv[:], smax_rep[0:1, 0:1])
                        ii.drain()
                        sv2 = VL(jconv[:], int(REVC) - N + 1, int(REVC))
                        jv = ii.compute_val(int(REVC) - sv2)
                        mb = VL(minv_sb[0:1, 0:1].bitcast(dt.int32))
                        ii.reg_save(minvlist[0:1, bass.ds(kv, 1)], mb)
                        # one-hot at j via is_equal(iotaJ, j): smax_rep holds
                        # (REVC - j) replicated, so jrep = REVC - smax_rep
                        ii.tensor_scalar(jrep[:], smax_rep[:], -1.0, REVC,
                                         AluOpType.mult, AluOpType.add)
                        ii.drain()
                        ii.tensor_scalar(ohtmp[:], iotaJf[:], jrep[:], None,
                                         AluOpType.is_equal)
                        ii.drain()
                        ii.tensor_scalar_mul(ohBIG[:], ohtmp[:], BIG)
                        ii.drain()
                        ii.tensor_tensor(v_eff[:], v_eff[:], ohBIG[:],
                                         AluOpType.subtract)
                        ii.tensor_tensor(spcm[:], spcm[:], ohBIG[:],
                                         AluOpType.add)
                        ii.drain()
                        rc = VL(row4col[g][0:1, bass.ds(jv, 1)], 0, M)
                        rc_reg = ii.to_reg(rc)
                        with ii.If_cmp(rc_reg, 0, "IS_EQ"):
                            ii.reg_mov(cont, 0)
                            r_jv = ii.to_reg(jv)
                            ii.reg_mov(sink_r, r_jv)
                        with ii.Else():
                            curn = ii.compute_val(rc - 1)
                            ii.reg_save(cur_sb[:], curn)
                            kn = ii.compute_val(kv + 1)
                            kn_reg = ii.to_reg(kn)
                            with ii.If_cmp(kn_reg, MAXIT, "IS_LT"):
                                ii.reg_save(k_sb[:], kn)
                                ii.reg_save(curlist[0:1, bass.ds(kn, 1)], curn)
                            with ii.Else():
                                ii.reg_mov(cont, 0)
                                ii.reg_save(ovf[0:1, g:g + 1], 1)
                    # ---- step end: duals ----
                    i_sv = ii.snap(i_r, min_val=0, max_val=M - 1)
                    ii.tensor_tensor(u_A[g][0:1, bass.ds(i_sv, 1)],
                                     u_A[g][0:1, bass.ds(i_sv, 1)],
                                     minv_sb[0:1, 0:1], AluOpType.add)
                    ii.drain()
                    # scanned-row duals: for t in 1..k: u[cur_t] += minv - minvlist[t-1]
                    ii.reg_save(t_sb[:], 1)
                    kfin = VL(k_sb[:], 0, MAXIT - 1)
                    kfin_reg = ii.to_reg(kfin, )
                    tcnt = ii.alloc_register(f"tcnt{g}")
                    ii.reg_mov(tcnt, kfin_reg)
                    with ii.While(lambda: tcnt):
                        tv = VL(t_sb[:], 1, MAXIT)
                        mtb = VL(minvlist[0:1, bass.ds(ii.compute_val(tv - 1), 1)])
                        ii.reg_save(mt_sb[:].bitcast(dt.int32), mtb)
                        rv = VL(curlist[0:1, bass.ds(tv, 1)], 0, M - 1)
                        ii.tensor_tensor(tdif[:], minv_sb[0:1, 0:1], mt_sb[:],
                                         AluOpType.subtract)
                        ii.drain()
                        ii.tensor_tensor(u_A[g][0:1, bass.ds(rv, 1)],
                                         u_A[g][0:1, bass.ds(rv, 1)],
                                         tdif[:], AluOpType.add)
                        ii.drain()
                        ii.reg_save(t_sb[:], ii.compute_val(tv + 1))
                        ii.reg_sub(tcnt, tcnt, 1)
                    # v update
                    ii.tensor_scalar(sc01[:], v_eff[:], -1e29, None,
                                     AluOpType.is_lt)
                    ii.drain()
                    ii.tensor_scalar(vsp[:], spc[:], gmin_rep[:], None,
                                     AluOpType.subtract)
                    ii.drain()
                    ii.tensor_tensor(vsp[:], vsp[:], sc01[:], AluOpType.mult)
                    ii.scalar_tensor_tensor(v_eff[:], sc01[:], BIG, v_eff[:],
                                            AluOpType.mult, AluOpType.add)
                    ii.drain()
                    ii.tensor_tensor(v_eff[:], v_eff[:], vsp[:], AluOpType.add)
                    ii.drain()
                    # ---- augmentation ----
                    ii.reg_save(j_sb[:], ii.snap(sink_r, min_val=0,
                                                 max_val=N - 1))
                    ii.reg_mov(cont2, 1)
                    with ii.While(lambda: cont2):
                        jv2 = VL(j_sb[:], 0, N - 1)
                        ii.reg_save(jfi[0:1, 0:1], jv2)
                        ii.tensor_copy(jfc[0:1, 0:1], jfi[0:1, 0:1])
                        ii.drain()
                        ii.stream_shuffle(jrep[:], jfc[:], BCAST0)
                        ii.drain()
                        ii.tensor_scalar(o2[:], iotaJf[:], jrep[:], None,
                                         AluOpType.is_equal)
                        ii.drain()
                        ii.tensor_tensor(se[:], stamp[:], o2[:], AluOpType.mult)
                        ii.drain()
                        ii.tensor_reduce(srow[:], se[:], AX.X, AluOpType.max)
                        ii.drain()
                        ii.tensor_copy(smax_rep[:], srow[:])
                        ii.drain()
                        butterfly(ii, smax_rep, AluOpType.max)
                        ii.tensor_copy(kconv[:], smax_rep[0:1, 0:1])
                        ii.drain()
                        kk = VL(kconv[:], 0, MAXIT - 1)
                        rv2 = VL(curlist[0:1, bass.ds(kk, 1)], 0, M - 1)
                        old = VL(col4row[g][0:1, bass.ds(rv2, 1)], 0, N)
                        ii.reg_save(row4col[g][0:1, bass.ds(jv2, 1)],
                                    ii.compute_val(rv2 + 1))
                        ii.reg_save(col4row[g][0:1, bass.ds(rv2, 1)],
                                    ii.compute_val(jv2 + 1))
                        rv2_reg = ii.to_reg(rv2)
                        with ii.If_cmp(rv2_reg, ii.snap(i_r, min_val=0,
                                                        max_val=M - 1),
                                       "IS_EQ"):
                            ii.reg_mov(cont2, 0)
                        with ii.Else():
                            ii.reg_save(j_sb[:], ii.compute_val(old - 1))
                # advance slot
                svn = VL(s_sb[:], 0, NRES - 1)
                ii.reg_save(s_sb[:], ii.compute_val(svn + 1))
                ii.reg_sub(sleft, sleft, 1)

    if dbg is not None and "c4r_final0" in dbg:
        for g in range(NIMG):
            nc.sync.dma_start(dbg[f"c4r_final{g}"], col4row[g][:])
        nc.sync.dma_start(dbg["ovf"], ovf[:])
    return dict(ovf=ovf)


def build_output(tc, ctx, outs, pool, C, S):
    """Phase 4: per-image rank sort + PE scatter -> out [NIMG, 2, 128] int32."""
    nc = tc.nc
    out_d = outs["out"]
    rowiotaF, iota128f, ones128, eye128 = (C["rowiotaF"], C["iota128f"],
                                           C["ones128"], C["eye128"])
    col4row = S["col4row"]

    opool = S["spool"]
    ops = ctx.enter_context(tc.tile_pool(name="outps", bufs=2, space="PSUM"))

    for g in range(NIMG):
        jrow = opool.tile([1, M], dt.float32, tag="jrow")
        nc.vector.tensor_copy(jrow[:], col4row[g][:])        # int -> f32 (j+1)
        nc.vector.tensor_scalar_add(jrow[:], jrow[:], -1.0)  # j
        jpp_ps = ops.tile([M, 1], dt.float32, tag="jpp_ps")
        nc.tensor.matmul(jpp_ps[:], jrow[:], S["ones11"][:])
        jpp = opool.tile([M, 1], dt.float32, tag="jpp")
        nc.vector.tensor_copy(jpp[:], jpp_ps[:])
        jb_ps = ops.tile([M, M], dt.float32, tag="jb_ps")
        nc.tensor.matmul(jb_ps[:], ones128[:], jrow[:])
        cmp = opool.tile([M, M], dt.float32, tag="cmp")
        nc.vector.tensor_scalar(cmp[:], jb_ps[:], jpp[:], None, AluOpType.is_lt)
        rank = opool.tile([M, 1], dt.float32, tag="rank")
        nc.vector.tensor_reduce(rank[:], cmp[:], AX.X, AluOpType.add)
        ohr = opool.tile([M, M], dt.float32, tag="ohr")
        nc.vector.tensor_scalar(ohr[:], iota128f[:], rank[:], None,
                                AluOpType.is_equal)
        pk2 = opool.tile([M, 2], dt.float32, tag="pk2")
        nc.vector.tensor_copy(pk2[:, 0:1], jpp[:])
        nc.vector.tensor_copy(pk2[:, 1:2], rowiotaF[:])
        om_ps = ops.tile([2, M], dt.float32, tag="om_ps")
        nc.tensor.matmul(om_ps[:], pk2[:], ohr[:])
        om = opool.tile([2, M], dt.int32, tag="om")
        nc.vector.tensor_copy(om[:], om_ps[:])
        nc.sync.dma_start(out_d[g], om[:])


_CACHE = {}


def _get_program():
    if "nc" in _CACHE:
        return _CACHE["nc"]
    import concourse.bacc as bacc
    import concourse.tile as tile
    from contextlib import ExitStack

    nc = bacc.Bacc("TRN2", target_bir_lowering=False, debug=False,
                   enable_asserts=False)
    ob = nc.dram_tensor("ob", [NIMG, N, 4], dt.float32, kind="ExternalInput").ap()
    tb = nc.dram_tensor("tb", [NIMG, M, 4], dt.float32, kind="ExternalInput").ap()
    out = nc.dram_tensor("out", [NIMG, 2, M], dt.int32, kind="ExternalOutput").ap()
    ins = {"ob": ob, "tb": tb}
    outs = {"out": out}
    with tile.TileContext(nc) as tc:
        with ExitStack() as ctx:
            pool, cpool, cost_A, C = build(tc, ctx, outs, ins)
            S = build_solve(tc, ctx, outs, ins, pool, cpool, cost_A, C)
            build_residual(tc, ctx, outs, ins, pool, cpool, cost_A, C, S)
            build_output(tc, ctx, outs, pool, C, S)
    nc.compile()
    _CACHE["nc"] = nc
    return nc


def kernel(out_boxes, tgt_boxes, _trace=False):
    from concourse.bass_utils import run_bass_kernel_spmd
    ob = np.ascontiguousarray(np.asarray(out_boxes, dtype=np.float32))
    tb = np.ascontiguousarray(np.asarray(tgt_boxes, dtype=np.float32))
    B = ob.shape[0]
    ncores = 8
    per = B // ncores
    nc = _get_program()
    in_maps = [{"ob": ob[c * per:(c + 1) * per], "tb": tb[c * per:(c + 1) * per]}
               for c in range(ncores)]
    res = run_bass_kernel_spmd(nc, in_maps, list(range(ncores)), trace=_trace)
    outp = np.concatenate([res.results[c]["out"] for c in range(ncores)], axis=0)
    if _trace:
        kernel.last_exec_time_ns = res.exec_time_ns
    return outp.astype(np.int32)

